# revision 11
# baseline (speedup 1.0000x reference)
"""Bidirectional LSTM encoder (B=32, T=256, E=300, H=512) on 8 TRN2 NeuronCores.

Sharding: data-parallel, core c in 0..3 -> forward direction, batch rows
8c..8c+8; core c in 4..7 -> backward direction (time-reversed inputs),
batch rows 8(c-4)..8(c-4)+8.  Embedding table and per-direction weights are
replicated to the cores that need them.

Per-core kernel: gather embedding rows (indirect DMA), transpose via PE,
precompute xW = [e,1] @ [W;b] into SBUF (bf16), then a 256-step recurrent
scan.  Each step computes z = xW_t + h @ U via 4 identity-matmuls (inject
xW_t into PSUM) + 16 U-matmuls (K-major so next step's matmuls are gated
per h^T-chunk), applies sigmoid/tanh on ACT (mask folded into per-partition
bias), gate math on DVE, masking blend on GPSIMD, and produces h^T for the
next step with DVE 32x32 block transposes.
"""

import os
import sys

for _p in ("/opt/trn_rl_repo",):
    if _p not in sys.path:
        sys.path.insert(0, _p)

import numpy as np
import ml_dtypes

BF16 = ml_dtypes.bfloat16

B, T, E, H, V = 32, 256, 300, 512, 50258
M = 8           # batch rows per core
NB = 16         # token blocks of 128 (M*T/128)
G4 = 4 * H      # 2048
EK = (128, 128, 65)   # e/W chunks (last = 44 dims + zero pad + bias row at 64)
BIG = 1.0e9

# Within-half column interleave: device column 64j+32k+c holds H-dim
# 128k+32j+c (k<2, j<4, c<32) so the per-step h^T build is four contiguous
# [32,64] DVE block-transposes per half.
_N256 = np.arange(256)
ILV = 128 * ((_N256 % 64) // 32) + 32 * (_N256 // 64) + (_N256 % 32)
# Column permutation: original U/W columns are [i | f | g | o] (512 each).
# New layout: [i_A o_A f_A g_A | i_B o_B f_B g_B] (256 each), each block
# ILV-interleaved.
_PERM = np.concatenate([
    base + 256 * half + ILV
    for half in (0, 1)
    for base in (0, 1536, 512, 1024)      # i, o, f, g
])
# device h/c column -> true H dim
_PERM_H = np.concatenate([ILV, 256 + ILV])
_INV_H = np.argsort(_PERM_H)

_COMPILED = None     # (nc, in_names) cache
LAST_RESULTS = None  # BassKernelResults of the most recent run (for tests)


def _install_ntff_hook_shim():
    """This image's antenv lacks axon_hooks; recreate it so trace=True can
    capture NTFF profiles via the axon .so (same recipe as trn_boot)."""
    import types, contextlib, ctypes
    try:
        from antenv.axon_hooks import get_axon_ntff_profile_hook  # noqa: F401
        return
    except ImportError:
        pass
    try:
        import antenv
    except ImportError:
        return
    mod = types.ModuleType("antenv.axon_hooks")
    _state = {"hook": None}
    def set_axon_ntff_profile_hook(h):
        _state["hook"] = h
    def get_axon_ntff_profile_hook():
        return _state["hook"]
    mod.set_axon_ntff_profile_hook = set_axon_ntff_profile_hook
    mod.get_axon_ntff_profile_hook = get_axon_ntff_profile_hook
    sys.modules["antenv.axon_hooks"] = mod
    antenv.axon_hooks = mod

    so_path = "/opt/axon/libaxon_pjrt.so"
    if not os.path.exists(so_path):
        return
    lib = ctypes.CDLL(so_path)
    if not hasattr(lib, "axon_start_nrt_profile"):
        return
    lib.axon_start_nrt_profile.argtypes = [ctypes.POINTER(ctypes.c_int64), ctypes.c_size_t]
    lib.axon_start_nrt_profile.restype = ctypes.c_int64
    lib.axon_stop_nrt_profile.argtypes = [ctypes.c_char_p]
    lib.axon_stop_nrt_profile.restype = ctypes.c_int64

    @contextlib.contextmanager
    def _hook(output_dir, device_ids):
        import jax
        jax.devices()
        if device_ids:
            ids = (ctypes.c_int64 * len(device_ids))(*device_ids)
            rc = lib.axon_start_nrt_profile(ids, len(device_ids))
        else:
            rc = lib.axon_start_nrt_profile(None, 0)
        if rc != 0:
            raise RuntimeError(f"axon_start_nrt_profile rc={rc}")
        try:
            yield
        finally:
            n = lib.axon_stop_nrt_profile(str(output_dir).encode())
            print(f"profile: {n} file(s) written to {output_dir}")

    set_axon_ntff_profile_hook(_hook)


def _build_nc(dbg=False):
    from contextlib import ExitStack
    from concourse import bass, bacc, mybir, tile

    f32 = mybir.dt.float32
    bf16 = mybir.dt.bfloat16
    i32 = mybir.dt.int32
    AF = mybir.ActivationFunctionType
    ALU = mybir.AluOpType

    nc = bacc.Bacc(
        "TRN2",
        target_bir_lowering=False,
        debug=False,
        enable_asserts=False,
        num_devices=8,
    )

    d_emb = nc.dram_tensor("emb", (V, E), f32, kind="ExternalInput")
    d_u = nc.dram_tensor("u_w", (4 * 128, G4), bf16, kind="ExternalInput")
    d_w = nc.dram_tensor("w_w", (3 * 128, G4), bf16, kind="ExternalInput")
    d_idx = nc.dram_tensor("idx", (128, NB), i32, kind="ExternalInput")
    d_bias_i = nc.dram_tensor("bias_i", (M, T), f32, kind="ExternalInput")
    d_bias_f = nc.dram_tensor("bias_f", (M, T), f32, kind="ExternalInput")
    d_ispad = nc.dram_tensor("ispad", (M, T), f32, kind="ExternalInput")
    d_hinit = nc.dram_tensor("h_init", (32, H), bf16, kind="ExternalInput")
    d_hTinit = nc.dram_tensor("hT_init", (128, 128), bf16, kind="ExternalInput")
    d_cinit = nc.dram_tensor("c_init", (M, H), bf16, kind="ExternalInput")
    d_idbf = nc.dram_tensor("ident_bf", (128, 128), bf16, kind="ExternalInput")
    d_idf32 = nc.dram_tensor("ident_f32", (128, 128), f32, kind="ExternalInput")

    if dbg:
        d_oxw = nc.dram_tensor("o_xw", (M * T, G4), bf16, kind="ExternalOutput")
        d_og = nc.dram_tensor("o_g", (M, G4), bf16, kind="ExternalOutput")
        d_oet = nc.dram_tensor("o_et", (128, 3 * G4), bf16, kind="ExternalOutput")
        d_oh1 = nc.dram_tensor("o_h1", (32, H), bf16, kind="ExternalOutput")
        d_oht1 = nc.dram_tensor("o_ht1", (128, 128), bf16, kind="ExternalOutput")
        d_oc1 = nc.dram_tensor("o_c1", (M, H), bf16, kind="ExternalOutput")
    d_oseq = nc.dram_tensor("o_seq", (M * T, H), bf16, kind="ExternalOutput")
    d_oh = nc.dram_tensor("o_h", (M, H), bf16, kind="ExternalOutput")
    d_oc = nc.dram_tensor("o_c", (M, H), bf16, kind="ExternalOutput")

    with ExitStack() as ctx:
        tc = ctx.enter_context(tile.TileContext(nc))
        const = ctx.enter_context(tc.tile_pool(name="const", bufs=1))

        u_sb = const.tile([128, 4 * G4], bf16, tag="u_sb")
        w_sb = const.tile([128, 3 * G4], bf16, tag="w_sb")
        et_sb = const.tile([128, 3 * G4], bf16, tag="et_sb")
        idx_sb = const.tile([128, NB], i32, tag="idx_sb")
        bias_i_sb = const.tile([M, T], f32, tag="bias_i_sb")
        bias_f_sb = const.tile([M, T], f32, tag="bias_f_sb")
        ispad_sb = const.tile([M, T], f32, tag="ispad_sb")
        idbf_sb = const.tile([128, 128], bf16, tag="idbf_sb")
        idf32_sb = const.tile([128, 128], f32, tag="idf32_sb")
        # state (explicit double buffers where needed)
        cA = const.tile([M, 256], bf16, tag="cA")
        cB = const.tile([M, 256], bf16, tag="cB")
        hA = [const.tile([32, 256], bf16, tag=f"hA{i}", name=f"hA{i}") for i in range(2)]
        hB = [const.tile([32, 256], bf16, tag=f"hB{i}", name=f"hB{i}") for i in range(2)]
        hTA = [const.tile([128, 64], bf16, tag=f"hTA{i}", name=f"hTA{i}") for i in range(2)]
        hTB = [const.tile([128, 64], bf16, tag=f"hTB{i}", name=f"hTB{i}") for i in range(2)]

        # ---- input loads ----
        nc.sync.dma_start(u_sb[:].rearrange("p (k n) -> p k n", k=4),
                          d_u.ap().rearrange("(k p) n -> p k n", p=128))
        nc.sync.dma_start(w_sb[:].rearrange("p (k n) -> p k n", k=3),
                          d_w.ap().rearrange("(k p) n -> p k n", p=128))
        nc.sync.dma_start(idx_sb[:], d_idx[:])
        nc.sync.dma_start(bias_i_sb[:], d_bias_i[:])
        nc.sync.dma_start(bias_f_sb[:], d_bias_f[:])
        nc.sync.dma_start(ispad_sb[:], d_ispad[:])
        nc.sync.dma_start(idbf_sb[:], d_idbf[:])
        nc.sync.dma_start(idf32_sb[:], d_idf32[:])
        nc.sync.dma_start(cA[:], d_cinit[:, 0:256])
        nc.sync.dma_start(cB[:], d_cinit[:, 256:512])
        nc.sync.dma_start(hA[0][:], d_hinit[:, 0:256])
        nc.sync.dma_start(hB[0][:], d_hinit[:, 256:512])
        nc.sync.dma_start(hTA[0][:], d_hTinit[:, 0:64])
        nc.sync.dma_start(hTB[0][:], d_hTinit[:, 64:128])
        nc.vector.memset(hA[1][:], 0.0)
        nc.vector.memset(hB[1][:], 0.0)
        # e^T chunk 2: zero the pad rows 44..63, ones-row at 64 (bias)
        nc.vector.memset(et_sb[32:64, 2 * G4:3 * G4], 0.0)
        nc.vector.memset(et_sb[64:128, 2 * G4:3 * G4], 1.0)

        # preload the sigmoid/tanh ACT table set early (off the scan path)
        warm = const.tile([1, 8], f32, tag="warm")
        nc.scalar.activation(warm[:], idf32_sb[0:1, 0:8], AF.Sigmoid)

        dramp = ctx.enter_context(tc.tile_pool(name="dramp", bufs=1, space="DRAM"))
        xw_d = dramp.tile([M * T, G4], bf16, tag="xw_d")

        # ---- phase 1: gather + transpose + xW precompute ----
        with tc.tile_pool(name="p1", bufs=3) as p1, \
             tc.tile_pool(name="p1ps", bufs=2, space="PSUM") as p1ps, \
             tc.tile_pool(name="p1ps2", bufs=1, space="PSUM") as p1ps2:
            for j in range(NB):
                etok = p1.tile([128, 304], f32, tag="etok")
                nc.gpsimd.indirect_dma_start(
                    out=etok[:, 0:E],
                    out_offset=None,
                    in_=d_emb[:],
                    in_offset=bass.IndirectOffsetOnAxis(ap=idx_sb[:, j:j + 1], axis=0),
                )
                for c in range(3):
                    cw = 128 if c < 2 else 44
                    tp = p1ps.tile([128, 128], f32, tag="tp")
                    nc.tensor.transpose(
                        out=tp[0:cw, 0:128],
                        in_=etok[0:128, c * 128:c * 128 + cw],
                        identity=idf32_sb[:],
                    )
                    nc.vector.tensor_copy(
                        et_sb[0:cw, c * G4 + j * 128:c * G4 + j * 128 + 128],
                        tp[0:cw, 0:128],
                    )
                xps = p1ps2.tile([128, G4], f32, tag="xps")
                for b4 in range(4):
                    for c in range(3):
                        kc = EK[c]
                        nc.tensor.matmul(
                            out=xps[:, b4 * 512:(b4 + 1) * 512],
                            lhsT=et_sb[0:kc, c * G4 + j * 128:c * G4 + j * 128 + 128],
                            rhs=w_sb[0:kc, c * G4 + b4 * 512:c * G4 + (b4 + 1) * 512],
                            start=(c == 0),
                            stop=(c == 2),
                        )
                xst = p1.tile([128, G4], bf16, tag="xst")
                if j % 2 == 0:
                    nc.vector.tensor_copy(xst[:], xps[:])
                else:
                    nc.scalar.copy(xst[:], xps[:])
                nc.sync.dma_start(xw_d[j * 128:(j + 1) * 128, :], xst[:])
                if dbg:
                    nc.sync.dma_start(d_oxw[j * 128:(j + 1) * 128, :], xst[:])

        # ---- phase 2: the scan ----
        zpA = ctx.enter_context(tc.tile_pool(name="zpA", bufs=2, space="PSUM"))
        zpB = ctx.enter_context(tc.tile_pool(name="zpB", bufs=2, space="PSUM"))
        gp = ctx.enter_context(tc.tile_pool(name="gp", bufs=2))
        xwp = ctx.enter_context(tc.tile_pool(name="xwp", bufs=4))
        tp2 = ctx.enter_context(tc.tile_pool(name="tp2", bufs=3))

        for t in range(T):
            r, w = t % 2, (t + 1) % 2

            zA = zpA.tile([32, 1024], mybir.dt.float32, tag="zA")
            zB = zpB.tile([32, 1024], mybir.dt.float32, tag="zB")
            xws = xwp.tile([M, G4], bf16, tag="xws")
            nc.sync.dma_start(xws[:], xw_d[t * M:(t + 1) * M, :])

            # PE: inject xW_t (identity matmul, start=True), then U-matmuls
            # K-major so each K chunk is gated only on its h^T chunk.
            for half, z in ((0, zA), (1, zB)):
                for b2 in range(2):
                    nc.tensor.matmul(
                        out=z[:, b2 * 512:(b2 + 1) * 512],
                        lhsT=idbf_sb[0:M, 0:32],
                        rhs=xws[0:M,
                                half * 1024 + b2 * 512:
                                half * 1024 + (b2 + 1) * 512],
                        start=True, stop=False,
                    )
            for k in range(4):
                hts = hTA[r] if k < 2 else hTB[r]
                col = (k % 2) * 32
                for half, z in ((0, zA), (1, zB)):
                    for b2 in range(2):
                        nc.tensor.matmul(
                            out=z[:, b2 * 512:(b2 + 1) * 512],
                            lhsT=hts[:, col:col + 32],
                            rhs=u_sb[:, k * G4 + half * 1024 + b2 * 512:
                                     k * G4 + half * 1024 + (b2 + 1) * 512],
                            start=False, stop=(k == 3),
                        )

            # ACT: gates.  Layout per half: [i(256) | o(256) | f(256) | g(256)]
            gA = gp.tile([M, 1024], bf16, tag="gA")
            gB = gp.tile([M, 1024], bf16, tag="gB")
            for z, g in ((zA, gA), (zB, gB)):
                nc.scalar.activation(g[:, 0:512], z[0:M, 0:512], AF.Sigmoid,
                                     bias=bias_i_sb[:, t:t + 1])
                nc.scalar.activation(g[:, 512:768], z[0:M, 512:768], AF.Sigmoid,
                                     bias=bias_f_sb[:, t:t + 1])
                nc.scalar.activation(g[:, 768:1024], z[0:M, 768:1024], AF.Tanh)

            if dbg and t == 0:
                nc.sync.dma_start(d_og[:, 0:1024], gA[:])
                nc.sync.dma_start(d_og[:, 1024:2048], gB[:])
                nc.sync.dma_start(d_oet[:], et_sb[:])

            # GPSIMD: masking keep-terms (ready early)
            hkA = tp2.tile([M, 256], bf16, tag="hkA")
            hkB = tp2.tile([M, 256], bf16, tag="hkB")
            nc.gpsimd.tensor_scalar(hkA[:], hA[r][0:M, :], ispad_sb[:, t:t + 1],
                                    None, op0=ALU.mult)
            nc.gpsimd.tensor_scalar(hkB[:], hB[r][0:M, :], ispad_sb[:, t:t + 1],
                                    None, op0=ALU.mult)

            # DVE: c update per half; then ot; transposes after h
            tcs = []
            for g, c_t, nm in ((gA, cA, "A"), (gB, cB, "B")):
                fc = tp2.tile([M, 256], bf16, tag=f"fc{nm}")
                ig = tp2.tile([M, 256], bf16, tag=f"ig{nm}")
                nc.vector.tensor_tensor(fc[:], g[:, 512:768], c_t[:], op=ALU.mult)
                nc.vector.tensor_tensor(ig[:], g[:, 0:256], g[:, 768:1024], op=ALU.mult)
                nc.vector.tensor_tensor(c_t[:], fc[:], ig[:], op=ALU.add)
                tcv = tp2.tile([M, 256], bf16, tag=f"tc{nm}")
                nc.scalar.activation(tcv[:], c_t[:], AF.Tanh)
                tcs.append(tcv)
            otA = tp2.tile([M, 256], bf16, tag="otA")
            otB = tp2.tile([M, 256], bf16, tag="otB")
            nc.vector.tensor_tensor(otA[:], gA[:, 256:512], tcs[0][:], op=ALU.mult)
            nc.vector.tensor_tensor(otB[:], gB[:, 256:512], tcs[1][:], op=ALU.mult)

            # GPSIMD: h_new = ot + hk   (masked rows: ot==0 -> h stays)
            nc.gpsimd.tensor_tensor(hA[w][0:M, :], otA[:], hkA[:], op=ALU.add)
            nc.gpsimd.tensor_tensor(hB[w][0:M, :], otB[:], hkB[:], op=ALU.add)

            # DVE: h^T via 32x32 block transposes (rows 8..31 are zero).
            # h cols are ILV-interleaved so each op is a contiguous [32,64].
            for hsrc, hdst in ((hA[w], hTA[w]), (hB[w], hTB[w])):
                for j in range(4):
                    nc.vector.transpose(
                        hdst[32 * j:32 * j + 32, :],
                        hsrc[:, 64 * j:64 * j + 64],
                    )

            if dbg and t == 0:
                nc.sync.dma_start(d_oh1[:, 0:256], hA[w][:])
                nc.sync.dma_start(d_oh1[:, 256:512], hB[w][:])
                nc.sync.dma_start(d_oht1[:, 0:64], hTA[w][:])
                nc.sync.dma_start(d_oht1[:, 64:128], hTB[w][:])
                nc.sync.dma_start(d_oc1[:, 0:256], cA[:])
                nc.sync.dma_start(d_oc1[:, 256:512], cB[:])

            # stream the step's output
            nc.sync.dma_start(d_oseq[t * M:(t + 1) * M, 0:256], hA[w][0:M, :])
            nc.sync.dma_start(d_oseq[t * M:(t + 1) * M, 256:512], hB[w][0:M, :])

        # final states
        wf = T % 2
        nc.sync.dma_start(d_oh[:, 0:256], hA[wf][0:M, :])
        nc.sync.dma_start(d_oh[:, 256:512], hB[wf][0:M, :])
        nc.sync.dma_start(d_oc[:, 0:256], cA[:])
        nc.sync.dma_start(d_oc[:, 256:512], cB[:])

    nc.compile()
    return nc


def _prep_core(x_sh, h0_sh, c0_sh, W, U, b):
    """Host-side layout prep for one core.  x_sh [M,T] int32 (already
    time-reversed for backward cores), h0/c0 [M,H] f32, W [E,4H], U [H,4H],
    b [4H] f32.  Returns the per-core in_map."""
    x_sh = np.ascontiguousarray(x_sh).astype(np.int32)
    # gather indices, token-major (token = t*M + b)
    flat = x_sh.T.reshape(-1)                       # [T*M]
    idx = flat.reshape(NB, 128).T.copy()            # [128, NB]
    ispad = (x_sh == 0).astype(np.float32)          # [M, T]
    bias_i = ispad * (-BIG)
    bias_f = ispad * BIG

    # W augmented with bias row, padded to 384 rows, columns permuted
    W_aug = np.zeros((384, G4), np.float32)
    W_aug[0:E] = W
    W_aug[320] = b        # chunk-2 row 64, pairs with the e^T ones-row
    W_perm = W_aug[:, _PERM].astype(BF16)
    U_perm = U[:, _PERM].astype(BF16)

    h_init = np.zeros((32, H), np.float32)
    h_init[0:M] = h0_sh[:, _PERM_H]
    hT_init = np.zeros((128, 128), np.float32)
    for k in range(4):
        hT_init[:, 32 * k:32 * k + M] = h0_sh[:, 128 * k:128 * (k + 1)].T

    return {
        "u_w": np.ascontiguousarray(U_perm),
        "w_w": np.ascontiguousarray(W_perm),
        "idx": np.ascontiguousarray(idx),
        "bias_i": np.ascontiguousarray(bias_i),
        "bias_f": np.ascontiguousarray(bias_f),
        "ispad": np.ascontiguousarray(ispad),
        "h_init": h_init.astype(BF16),
        "hT_init": hT_init.astype(BF16),
        "c_init": c0_sh[:, _PERM_H].astype(BF16),
        "ident_bf": np.eye(128, dtype=BF16),
        "ident_f32": np.eye(128, dtype=np.float32),
    }


def kernel(x, h0_fwd, c0_fwd, h0_bwd, c0_bwd, emb,
           W_fwd, U_fwd, b_fwd, W_bwd, U_bwd, b_bwd):
    global _COMPILED, LAST_RESULTS
    from concourse.bass_utils import run_bass_kernel_spmd

    x = np.asarray(x, np.int32)
    emb = np.ascontiguousarray(np.asarray(emb, np.float32))
    h0_fwd = np.asarray(h0_fwd, np.float32)
    c0_fwd = np.asarray(c0_fwd, np.float32)
    h0_bwd = np.asarray(h0_bwd, np.float32)
    c0_bwd = np.asarray(c0_bwd, np.float32)
    W_fwd, U_fwd, b_fwd = (np.asarray(a, np.float32) for a in (W_fwd, U_fwd, b_fwd))
    W_bwd, U_bwd, b_bwd = (np.asarray(a, np.float32) for a in (W_bwd, U_bwd, b_bwd))

    if _COMPILED is None:
        _COMPILED = _build_nc()
    nc = _COMPILED

    in_maps = []
    for core in range(8):
        if core < 4:
            rows = slice(8 * core, 8 * core + 8)
            m = _prep_core(x[rows], h0_fwd[rows], c0_fwd[rows], W_fwd, U_fwd, b_fwd)
        else:
            rows = slice(8 * (core - 4), 8 * (core - 4) + 8)
            m = _prep_core(x[rows, ::-1], h0_bwd[rows], c0_bwd[rows],
                           W_bwd, U_bwd, b_bwd)
        m["emb"] = emb
        in_maps.append(m)

    trace = bool(int(os.environ.get("KERNEL_TRACE", "0")))
    if trace:
        _install_ntff_hook_shim()
    res = run_bass_kernel_spmd(
        nc, in_maps, core_ids=list(range(8)), trace=trace,
    )
    LAST_RESULTS = res

    output = np.zeros((B, T, 2 * H), np.float32)
    h_f = np.zeros((B, H), np.float32)
    h_b = np.zeros((B, H), np.float32)
    c_f = np.zeros((B, H), np.float32)
    c_b = np.zeros((B, H), np.float32)
    for core in range(8):
        r = res.results[core]
        seq = np.asarray(r["o_seq"]).astype(np.float32)      # [T*M, H]
        seq = seq.reshape(T, M, H).transpose(1, 0, 2)[:, :, _INV_H]
        hT_ = np.asarray(r["o_h"]).astype(np.float32)[:, _INV_H]
        cT_ = np.asarray(r["o_c"]).astype(np.float32)[:, _INV_H]
        if core < 4:
            rows = slice(8 * core, 8 * core + 8)
            output[rows, :, 0:H] = seq
            h_f[rows] = hT_
            c_f[rows] = cT_
        else:
            rows = slice(8 * (core - 4), 8 * (core - 4) + 8)
            output[rows, :, H:2 * H] = seq[:, ::-1, :]
            h_b[rows] = hT_
            c_b[rows] = cT_
    return output, h_f, h_b, c_f, c_b


# revision 12
# speedup vs baseline: 1.0175x; 1.0175x over previous
"""Bidirectional LSTM encoder (B=32, T=256, E=300, H=512) on 8 TRN2 NeuronCores.

Sharding: data-parallel, core c in 0..3 -> forward direction, batch rows
8c..8c+8; core c in 4..7 -> backward direction (time-reversed inputs),
batch rows 8(c-4)..8(c-4)+8.  Embedding table and per-direction weights are
replicated to the cores that need them.

Per-core kernel: gather embedding rows (indirect DMA), transpose via PE,
precompute xW = [e,1] @ [W;b] into SBUF (bf16), then a 256-step recurrent
scan.  Each step computes z = xW_t + h @ U via 4 identity-matmuls (inject
xW_t into PSUM) + 16 U-matmuls (K-major so next step's matmuls are gated
per h^T-chunk), applies sigmoid/tanh on ACT (mask folded into per-partition
bias), gate math on DVE, masking blend on GPSIMD, and produces h^T for the
next step with DVE 32x32 block transposes.
"""

import os
import sys

for _p in ("/opt/trn_rl_repo",):
    if _p not in sys.path:
        sys.path.insert(0, _p)

import numpy as np
import ml_dtypes

BF16 = ml_dtypes.bfloat16

B, T, E, H, V = 32, 256, 300, 512, 50258
M = 8           # batch rows per core
NB = 16         # token blocks of 128 (M*T/128)
G4 = 4 * H      # 2048
EK = (128, 128, 65)   # e/W chunks (last = 44 dims + zero pad + bias row at 64)
BIG = 1.0e9

# Within-half column interleave: device column 64j+32k+c holds H-dim
# 128k+32j+c (k<2, j<4, c<32) so the per-step h^T build is four contiguous
# [32,64] DVE block-transposes per half.
_N256 = np.arange(256)
ILV = 128 * ((_N256 % 64) // 32) + 32 * (_N256 // 64) + (_N256 % 32)
# Column permutation: original U/W columns are [i | f | g | o] (512 each).
# New layout: [i_A o_A f_A g_A | i_B o_B f_B g_B] (256 each), each block
# ILV-interleaved.
_PERM = np.concatenate([
    base + 256 * half + ILV
    for half in (0, 1)
    for base in (0, 1536, 512, 1024)      # i, o, f, g
])
# device h/c column -> true H dim
_PERM_H = np.concatenate([ILV, 256 + ILV])
_INV_H = np.argsort(_PERM_H)

_COMPILED = None     # (nc, in_names) cache
LAST_RESULTS = None  # BassKernelResults of the most recent run (for tests)


def _install_ntff_hook_shim():
    """This image's antenv lacks axon_hooks; recreate it so trace=True can
    capture NTFF profiles via the axon .so (same recipe as trn_boot)."""
    import types, contextlib, ctypes
    try:
        from antenv.axon_hooks import get_axon_ntff_profile_hook  # noqa: F401
        return
    except ImportError:
        pass
    try:
        import antenv
    except ImportError:
        return
    mod = types.ModuleType("antenv.axon_hooks")
    _state = {"hook": None}
    def set_axon_ntff_profile_hook(h):
        _state["hook"] = h
    def get_axon_ntff_profile_hook():
        return _state["hook"]
    mod.set_axon_ntff_profile_hook = set_axon_ntff_profile_hook
    mod.get_axon_ntff_profile_hook = get_axon_ntff_profile_hook
    sys.modules["antenv.axon_hooks"] = mod
    antenv.axon_hooks = mod

    so_path = "/opt/axon/libaxon_pjrt.so"
    if not os.path.exists(so_path):
        return
    lib = ctypes.CDLL(so_path)
    if not hasattr(lib, "axon_start_nrt_profile"):
        return
    lib.axon_start_nrt_profile.argtypes = [ctypes.POINTER(ctypes.c_int64), ctypes.c_size_t]
    lib.axon_start_nrt_profile.restype = ctypes.c_int64
    lib.axon_stop_nrt_profile.argtypes = [ctypes.c_char_p]
    lib.axon_stop_nrt_profile.restype = ctypes.c_int64

    @contextlib.contextmanager
    def _hook(output_dir, device_ids):
        import jax
        jax.devices()
        if device_ids:
            ids = (ctypes.c_int64 * len(device_ids))(*device_ids)
            rc = lib.axon_start_nrt_profile(ids, len(device_ids))
        else:
            rc = lib.axon_start_nrt_profile(None, 0)
        if rc != 0:
            raise RuntimeError(f"axon_start_nrt_profile rc={rc}")
        try:
            yield
        finally:
            n = lib.axon_stop_nrt_profile(str(output_dir).encode())
            print(f"profile: {n} file(s) written to {output_dir}")

    set_axon_ntff_profile_hook(_hook)


def _build_nc(dbg=False):
    from contextlib import ExitStack
    from concourse import bass, bacc, mybir, tile

    f32 = mybir.dt.float32
    bf16 = mybir.dt.bfloat16
    i32 = mybir.dt.int32
    AF = mybir.ActivationFunctionType
    ALU = mybir.AluOpType

    nc = bacc.Bacc(
        "TRN2",
        target_bir_lowering=False,
        debug=False,
        enable_asserts=False,
        num_devices=8,
    )

    d_emb = nc.dram_tensor("emb", (V, E), f32, kind="ExternalInput")
    d_u = nc.dram_tensor("u_w", (4 * 128, G4), bf16, kind="ExternalInput")
    d_w = nc.dram_tensor("w_w", (3 * 128, G4), bf16, kind="ExternalInput")
    d_idx = nc.dram_tensor("idx", (128, NB), i32, kind="ExternalInput")
    d_bias_i = nc.dram_tensor("bias_i", (M, T), f32, kind="ExternalInput")
    d_bias_f = nc.dram_tensor("bias_f", (M, T), f32, kind="ExternalInput")
    d_ispad = nc.dram_tensor("ispad", (M, T), f32, kind="ExternalInput")
    d_hinit = nc.dram_tensor("h_init", (32, H), bf16, kind="ExternalInput")
    d_hTinit = nc.dram_tensor("hT_init", (128, 128), bf16, kind="ExternalInput")
    d_cinit = nc.dram_tensor("c_init", (M, H), bf16, kind="ExternalInput")
    d_idbf = nc.dram_tensor("ident_bf", (128, 128), bf16, kind="ExternalInput")
    d_idf32 = nc.dram_tensor("ident_f32", (128, 128), f32, kind="ExternalInput")

    if dbg:
        d_oxw = nc.dram_tensor("o_xw", (M * T, G4), bf16, kind="ExternalOutput")
        d_og = nc.dram_tensor("o_g", (M, G4), bf16, kind="ExternalOutput")
        d_oet = nc.dram_tensor("o_et", (128, 3 * G4), bf16, kind="ExternalOutput")
        d_oh1 = nc.dram_tensor("o_h1", (32, H), bf16, kind="ExternalOutput")
        d_oht1 = nc.dram_tensor("o_ht1", (128, 128), bf16, kind="ExternalOutput")
        d_oc1 = nc.dram_tensor("o_c1", (M, H), bf16, kind="ExternalOutput")
    d_oseq = nc.dram_tensor("o_seq", (M * T, H), bf16, kind="ExternalOutput")
    d_oh = nc.dram_tensor("o_h", (M, H), bf16, kind="ExternalOutput")
    d_oc = nc.dram_tensor("o_c", (M, H), bf16, kind="ExternalOutput")

    with ExitStack() as ctx:
        tc = ctx.enter_context(tile.TileContext(nc))
        const = ctx.enter_context(tc.tile_pool(name="const", bufs=1))

        u_sb = const.tile([128, 4 * G4], bf16, tag="u_sb")
        w_sb = const.tile([128, 3 * G4], bf16, tag="w_sb")
        et_sb = const.tile([128, 3 * G4], bf16, tag="et_sb")
        idx_sb = const.tile([128, NB], i32, tag="idx_sb")
        bias_i_sb = const.tile([M, T], f32, tag="bias_i_sb")
        bias_f_sb = const.tile([M, T], f32, tag="bias_f_sb")
        ispad_sb = const.tile([M, T], f32, tag="ispad_sb")
        idbf_sb = const.tile([128, 128], bf16, tag="idbf_sb")
        idf32_sb = const.tile([128, 128], f32, tag="idf32_sb")
        # state (explicit double buffers where needed)
        cA = const.tile([M, 256], bf16, tag="cA")
        cB = const.tile([M, 256], bf16, tag="cB")
        hA = [const.tile([32, 256], bf16, tag=f"hA{i}", name=f"hA{i}") for i in range(2)]
        hB = [const.tile([32, 256], bf16, tag=f"hB{i}", name=f"hB{i}") for i in range(2)]
        hTA = [const.tile([128, 64], bf16, tag=f"hTA{i}", name=f"hTA{i}") for i in range(2)]
        hTB = [const.tile([128, 64], bf16, tag=f"hTB{i}", name=f"hTB{i}") for i in range(2)]

        # ---- input loads ----
        nc.sync.dma_start(u_sb[:].rearrange("p (k n) -> p k n", k=4),
                          d_u.ap().rearrange("(k p) n -> p k n", p=128))
        nc.sync.dma_start(w_sb[:].rearrange("p (k n) -> p k n", k=3),
                          d_w.ap().rearrange("(k p) n -> p k n", p=128))
        nc.sync.dma_start(idx_sb[:], d_idx[:])
        nc.sync.dma_start(bias_i_sb[:], d_bias_i[:])
        nc.sync.dma_start(bias_f_sb[:], d_bias_f[:])
        nc.sync.dma_start(ispad_sb[:], d_ispad[:])
        nc.sync.dma_start(idbf_sb[:], d_idbf[:])
        nc.sync.dma_start(idf32_sb[:], d_idf32[:])
        nc.sync.dma_start(cA[:], d_cinit[:, 0:256])
        nc.sync.dma_start(cB[:], d_cinit[:, 256:512])
        nc.sync.dma_start(hA[0][:], d_hinit[:, 0:256])
        nc.sync.dma_start(hB[0][:], d_hinit[:, 256:512])
        nc.sync.dma_start(hTA[0][:], d_hTinit[:, 0:64])
        nc.sync.dma_start(hTB[0][:], d_hTinit[:, 64:128])
        nc.vector.memset(hA[1][:], 0.0)
        nc.vector.memset(hB[1][:], 0.0)
        # e^T chunk 2: zero the pad rows 44..63, ones-row at 64 (bias)
        nc.vector.memset(et_sb[32:64, 2 * G4:3 * G4], 0.0)
        nc.vector.memset(et_sb[64:128, 2 * G4:3 * G4], 1.0)

        # preload the sigmoid/tanh ACT table set early (off the scan path)
        warm = const.tile([1, 8], f32, tag="warm")
        nc.scalar.activation(warm[:], idf32_sb[0:1, 0:8], AF.Sigmoid)

        dramp = ctx.enter_context(tc.tile_pool(name="dramp", bufs=1, space="DRAM"))
        xw_d = dramp.tile([M * T, G4], bf16, tag="xw_d")

        # ---- phase 1: gather + transpose + xW precompute ----
        with tc.tile_pool(name="p1", bufs=3) as p1, \
             tc.tile_pool(name="p1ps", bufs=2, space="PSUM") as p1ps, \
             tc.tile_pool(name="p1ps2", bufs=1, space="PSUM") as p1ps2:
            for j in range(NB):
                etok = p1.tile([128, 304], f32, tag="etok")
                nc.gpsimd.indirect_dma_start(
                    out=etok[:, 0:E],
                    out_offset=None,
                    in_=d_emb[:],
                    in_offset=bass.IndirectOffsetOnAxis(ap=idx_sb[:, j:j + 1], axis=0),
                )
                for c in range(3):
                    cw = 128 if c < 2 else 44
                    tp = p1ps.tile([128, 128], f32, tag="tp")
                    nc.tensor.transpose(
                        out=tp[0:cw, 0:128],
                        in_=etok[0:128, c * 128:c * 128 + cw],
                        identity=idf32_sb[:],
                    )
                    nc.vector.tensor_copy(
                        et_sb[0:cw, c * G4 + j * 128:c * G4 + j * 128 + 128],
                        tp[0:cw, 0:128],
                    )
                xps = p1ps2.tile([128, G4], f32, tag="xps")
                for b4 in range(4):
                    for c in range(3):
                        kc = EK[c]
                        nc.tensor.matmul(
                            out=xps[:, b4 * 512:(b4 + 1) * 512],
                            lhsT=et_sb[0:kc, c * G4 + j * 128:c * G4 + j * 128 + 128],
                            rhs=w_sb[0:kc, c * G4 + b4 * 512:c * G4 + (b4 + 1) * 512],
                            start=(c == 0),
                            stop=(c == 2),
                        )
                xst = p1.tile([128, G4], bf16, tag="xst")
                if j % 2 == 0:
                    nc.vector.tensor_copy(xst[:], xps[:])
                else:
                    nc.scalar.copy(xst[:], xps[:])
                nc.sync.dma_start(xw_d[j * 128:(j + 1) * 128, :], xst[:])
                if dbg:
                    nc.sync.dma_start(d_oxw[j * 128:(j + 1) * 128, :], xst[:])

        # ---- phase 2: the scan ----
        zpA = ctx.enter_context(tc.tile_pool(name="zpA", bufs=2, space="PSUM"))
        zpB = ctx.enter_context(tc.tile_pool(name="zpB", bufs=2, space="PSUM"))
        gp = ctx.enter_context(tc.tile_pool(name="gp", bufs=2))
        xwp = ctx.enter_context(tc.tile_pool(name="xwp", bufs=4))
        tp2 = ctx.enter_context(tc.tile_pool(name="tp2", bufs=3))

        for t in range(T):
            r, w = t % 2, (t + 1) % 2

            zA = zpA.tile([32, 1024], mybir.dt.float32, tag="zA")
            zB = zpB.tile([32, 1024], mybir.dt.float32, tag="zB")
            xws = xwp.tile([M, G4], bf16, tag="xws")
            nc.sync.dma_start(xws[:], xw_d[t * M:(t + 1) * M, :])

            # PE: inject xW_t (identity matmul, start=True), then U-matmuls
            # K-major so each K chunk is gated only on its h^T chunk.
            for half, z in ((0, zA), (1, zB)):
                for b2 in range(2):
                    nc.tensor.matmul(
                        out=z[:, b2 * 512:(b2 + 1) * 512],
                        lhsT=idbf_sb[0:M, 0:32],
                        rhs=xws[0:M,
                                half * 1024 + b2 * 512:
                                half * 1024 + (b2 + 1) * 512],
                        start=True, stop=False,
                    )
            for k in range(4):
                hts = hTA[r] if k < 2 else hTB[r]
                col = (k % 2) * 32
                for half, z in ((0, zA), (1, zB)):
                    for b2 in range(2):
                        nc.tensor.matmul(
                            out=z[:, b2 * 512:(b2 + 1) * 512],
                            lhsT=hts[:, col:col + 32],
                            rhs=u_sb[:, k * G4 + half * 1024 + b2 * 512:
                                     k * G4 + half * 1024 + (b2 + 1) * 512],
                            start=False, stop=(k == 3),
                        )

            # ACT: gates.  Layout per half: [i(256) | o(256) | f(256) | g(256)]
            gA = gp.tile([M, 1024], bf16, tag="gA")
            gB = gp.tile([M, 1024], bf16, tag="gB")
            for z, g in ((zA, gA), (zB, gB)):
                nc.scalar.activation(g[:, 0:512], z[0:M, 0:512], AF.Sigmoid,
                                     bias=bias_i_sb[:, t:t + 1])
                nc.scalar.activation(g[:, 512:768], z[0:M, 512:768], AF.Sigmoid,
                                     bias=bias_f_sb[:, t:t + 1])
                nc.scalar.activation(g[:, 768:1024], z[0:M, 768:1024], AF.Tanh)

            if dbg and t == 0:
                nc.sync.dma_start(d_og[:, 0:1024], gA[:])
                nc.sync.dma_start(d_og[:, 1024:2048], gB[:])
                nc.sync.dma_start(d_oet[:], et_sb[:])

            # c update: fc on GPSIMD, ig/c-add on DVE
            tcs = []
            for g, c_t, nm in ((gA, cA, "A"), (gB, cB, "B")):
                fc = tp2.tile([M, 256], bf16, tag=f"fc{nm}")
                ig = tp2.tile([M, 256], bf16, tag=f"ig{nm}")
                nc.gpsimd.tensor_tensor(fc[:], g[:, 512:768], c_t[:], op=ALU.mult)
                nc.vector.tensor_tensor(ig[:], g[:, 0:256], g[:, 768:1024], op=ALU.mult)
                nc.vector.tensor_tensor(c_t[:], fc[:], ig[:], op=ALU.add)
                tcv = tp2.tile([M, 256], bf16, tag=f"tc{nm}")
                nc.scalar.activation(tcv[:], c_t[:], AF.Tanh)
                tcs.append(tcv)
            otA = tp2.tile([M, 256], bf16, tag="otA")
            otB = tp2.tile([M, 256], bf16, tag="otB")
            nc.gpsimd.tensor_tensor(otA[:], gA[:, 256:512], tcs[0][:], op=ALU.mult)
            nc.gpsimd.tensor_tensor(otB[:], gB[:, 256:512], tcs[1][:], op=ALU.mult)

            # h_new = h_old * is_pad + ot, fused on DVE
            # (masked rows: ot==0 and is_pad==1 -> h carries)
            nc.vector.scalar_tensor_tensor(
                hA[w][0:M, :], hA[r][0:M, :], ispad_sb[:, t:t + 1], otA[:],
                op0=ALU.mult, op1=ALU.add)
            nc.vector.scalar_tensor_tensor(
                hB[w][0:M, :], hB[r][0:M, :], ispad_sb[:, t:t + 1], otB[:],
                op0=ALU.mult, op1=ALU.add)

            # DVE: h^T via 32x32 block transposes (rows 8..31 are zero).
            # h cols are ILV-interleaved so each op is a contiguous [32,64].
            for hsrc, hdst in ((hA[w], hTA[w]), (hB[w], hTB[w])):
                for j in range(4):
                    nc.vector.transpose(
                        hdst[32 * j:32 * j + 32, :],
                        hsrc[:, 64 * j:64 * j + 64],
                    )

            if dbg and t == 0:
                nc.sync.dma_start(d_oh1[:, 0:256], hA[w][:])
                nc.sync.dma_start(d_oh1[:, 256:512], hB[w][:])
                nc.sync.dma_start(d_oht1[:, 0:64], hTA[w][:])
                nc.sync.dma_start(d_oht1[:, 64:128], hTB[w][:])
                nc.sync.dma_start(d_oc1[:, 0:256], cA[:])
                nc.sync.dma_start(d_oc1[:, 256:512], cB[:])

            # stream the step's output
            nc.sync.dma_start(d_oseq[t * M:(t + 1) * M, 0:256], hA[w][0:M, :])
            nc.sync.dma_start(d_oseq[t * M:(t + 1) * M, 256:512], hB[w][0:M, :])

        # final states
        wf = T % 2
        nc.sync.dma_start(d_oh[:, 0:256], hA[wf][0:M, :])
        nc.sync.dma_start(d_oh[:, 256:512], hB[wf][0:M, :])
        nc.sync.dma_start(d_oc[:, 0:256], cA[:])
        nc.sync.dma_start(d_oc[:, 256:512], cB[:])

    nc.compile()
    return nc


def _prep_core(x_sh, h0_sh, c0_sh, W, U, b):
    """Host-side layout prep for one core.  x_sh [M,T] int32 (already
    time-reversed for backward cores), h0/c0 [M,H] f32, W [E,4H], U [H,4H],
    b [4H] f32.  Returns the per-core in_map."""
    x_sh = np.ascontiguousarray(x_sh).astype(np.int32)
    # gather indices, token-major (token = t*M + b)
    flat = x_sh.T.reshape(-1)                       # [T*M]
    idx = flat.reshape(NB, 128).T.copy()            # [128, NB]
    ispad = (x_sh == 0).astype(np.float32)          # [M, T]
    bias_i = ispad * (-BIG)
    bias_f = ispad * BIG

    # W augmented with bias row, padded to 384 rows, columns permuted
    W_aug = np.zeros((384, G4), np.float32)
    W_aug[0:E] = W
    W_aug[320] = b        # chunk-2 row 64, pairs with the e^T ones-row
    W_perm = W_aug[:, _PERM].astype(BF16)
    U_perm = U[:, _PERM].astype(BF16)

    h_init = np.zeros((32, H), np.float32)
    h_init[0:M] = h0_sh[:, _PERM_H]
    hT_init = np.zeros((128, 128), np.float32)
    for k in range(4):
        hT_init[:, 32 * k:32 * k + M] = h0_sh[:, 128 * k:128 * (k + 1)].T

    return {
        "u_w": np.ascontiguousarray(U_perm),
        "w_w": np.ascontiguousarray(W_perm),
        "idx": np.ascontiguousarray(idx),
        "bias_i": np.ascontiguousarray(bias_i),
        "bias_f": np.ascontiguousarray(bias_f),
        "ispad": np.ascontiguousarray(ispad),
        "h_init": h_init.astype(BF16),
        "hT_init": hT_init.astype(BF16),
        "c_init": c0_sh[:, _PERM_H].astype(BF16),
        "ident_bf": np.eye(128, dtype=BF16),
        "ident_f32": np.eye(128, dtype=np.float32),
    }


def kernel(x, h0_fwd, c0_fwd, h0_bwd, c0_bwd, emb,
           W_fwd, U_fwd, b_fwd, W_bwd, U_bwd, b_bwd):
    global _COMPILED, LAST_RESULTS
    from concourse.bass_utils import run_bass_kernel_spmd

    x = np.asarray(x, np.int32)
    emb = np.ascontiguousarray(np.asarray(emb, np.float32))
    h0_fwd = np.asarray(h0_fwd, np.float32)
    c0_fwd = np.asarray(c0_fwd, np.float32)
    h0_bwd = np.asarray(h0_bwd, np.float32)
    c0_bwd = np.asarray(c0_bwd, np.float32)
    W_fwd, U_fwd, b_fwd = (np.asarray(a, np.float32) for a in (W_fwd, U_fwd, b_fwd))
    W_bwd, U_bwd, b_bwd = (np.asarray(a, np.float32) for a in (W_bwd, U_bwd, b_bwd))

    if _COMPILED is None:
        _COMPILED = _build_nc()
    nc = _COMPILED

    in_maps = []
    for core in range(8):
        if core < 4:
            rows = slice(8 * core, 8 * core + 8)
            m = _prep_core(x[rows], h0_fwd[rows], c0_fwd[rows], W_fwd, U_fwd, b_fwd)
        else:
            rows = slice(8 * (core - 4), 8 * (core - 4) + 8)
            m = _prep_core(x[rows, ::-1], h0_bwd[rows], c0_bwd[rows],
                           W_bwd, U_bwd, b_bwd)
        m["emb"] = emb
        in_maps.append(m)

    trace = bool(int(os.environ.get("KERNEL_TRACE", "0")))
    if trace:
        _install_ntff_hook_shim()
    res = run_bass_kernel_spmd(
        nc, in_maps, core_ids=list(range(8)), trace=trace,
    )
    LAST_RESULTS = res

    output = np.zeros((B, T, 2 * H), np.float32)
    h_f = np.zeros((B, H), np.float32)
    h_b = np.zeros((B, H), np.float32)
    c_f = np.zeros((B, H), np.float32)
    c_b = np.zeros((B, H), np.float32)
    for core in range(8):
        r = res.results[core]
        seq = np.asarray(r["o_seq"]).astype(np.float32)      # [T*M, H]
        seq = seq.reshape(T, M, H).transpose(1, 0, 2)[:, :, _INV_H]
        hT_ = np.asarray(r["o_h"]).astype(np.float32)[:, _INV_H]
        cT_ = np.asarray(r["o_c"]).astype(np.float32)[:, _INV_H]
        if core < 4:
            rows = slice(8 * core, 8 * core + 8)
            output[rows, :, 0:H] = seq
            h_f[rows] = hT_
            c_f[rows] = cT_
        else:
            rows = slice(8 * (core - 4), 8 * (core - 4) + 8)
            output[rows, :, H:2 * H] = seq[:, ::-1, :]
            h_b[rows] = hT_
            c_b[rows] = cT_
    return output, h_f, h_b, c_f, c_b


# revision 13
# speedup vs baseline: 1.2599x; 1.2382x over previous
"""Bidirectional LSTM encoder (B=32, T=256, E=300, H=512) on 8 TRN2 NeuronCores.

Sharding: data-parallel, core c in 0..3 -> forward direction, batch rows
8c..8c+8; core c in 4..7 -> backward direction (time-reversed inputs),
batch rows 8(c-4)..8(c-4)+8.  Embedding table and per-direction weights are
replicated to the cores that need them.

Per-core kernel: gather embedding rows (indirect DMA), transpose via PE,
precompute xW = [e,1] @ [W;b] into SBUF (bf16), then a 256-step recurrent
scan.  Each step computes z = xW_t + h @ U via 4 identity-matmuls (inject
xW_t into PSUM) + 16 U-matmuls (K-major so next step's matmuls are gated
per h^T-chunk), applies sigmoid/tanh on ACT (mask folded into per-partition
bias), gate math on DVE, masking blend on GPSIMD, and produces h^T for the
next step with DVE 32x32 block transposes.
"""

import os
import sys

for _p in ("/opt/trn_rl_repo",):
    if _p not in sys.path:
        sys.path.insert(0, _p)

import numpy as np
import ml_dtypes

BF16 = ml_dtypes.bfloat16

B, T, E, H, V = 32, 256, 300, 512, 50258
M = 8           # batch rows per core
NB = 16         # token blocks of 128 (M*T/128)
G4 = 4 * H      # 2048
EK = (128, 128, 65)   # e/W chunks (last = 44 dims + zero pad + bias row at 64)
BIG = 1.0e9

# Within-half column interleave: device column 64j+32k+c holds H-dim
# 128k+32j+c (k<2, j<4, c<32) so the per-step h^T build is four contiguous
# [32,64] DVE block-transposes per half.
_N256 = np.arange(256)
ILV = 128 * ((_N256 % 64) // 32) + 32 * (_N256 // 64) + (_N256 % 32)
# Column permutation: original U/W columns are [i | f | g | o] (512 each).
# New layout: [i_A o_A f_A g_A | i_B o_B f_B g_B] (256 each), each block
# ILV-interleaved.
_PERM = np.concatenate([
    base + 256 * half + ILV
    for half in (0, 1)
    for base in (0, 1536, 512, 1024)      # i, o, f, g
])
# device h/c column -> true H dim
_PERM_H = np.concatenate([ILV, 256 + ILV])
_INV_H = np.argsort(_PERM_H)

_COMPILED = None     # (nc, in_names) cache
LAST_RESULTS = None  # BassKernelResults of the most recent run (for tests)


def _install_ntff_hook_shim():
    """This image's antenv lacks axon_hooks; recreate it so trace=True can
    capture NTFF profiles via the axon .so (same recipe as trn_boot)."""
    import types, contextlib, ctypes
    try:
        from antenv.axon_hooks import get_axon_ntff_profile_hook  # noqa: F401
        return
    except ImportError:
        pass
    try:
        import antenv
    except ImportError:
        return
    mod = types.ModuleType("antenv.axon_hooks")
    _state = {"hook": None}
    def set_axon_ntff_profile_hook(h):
        _state["hook"] = h
    def get_axon_ntff_profile_hook():
        return _state["hook"]
    mod.set_axon_ntff_profile_hook = set_axon_ntff_profile_hook
    mod.get_axon_ntff_profile_hook = get_axon_ntff_profile_hook
    sys.modules["antenv.axon_hooks"] = mod
    antenv.axon_hooks = mod

    so_path = "/opt/axon/libaxon_pjrt.so"
    if not os.path.exists(so_path):
        return
    lib = ctypes.CDLL(so_path)
    if not hasattr(lib, "axon_start_nrt_profile"):
        return
    lib.axon_start_nrt_profile.argtypes = [ctypes.POINTER(ctypes.c_int64), ctypes.c_size_t]
    lib.axon_start_nrt_profile.restype = ctypes.c_int64
    lib.axon_stop_nrt_profile.argtypes = [ctypes.c_char_p]
    lib.axon_stop_nrt_profile.restype = ctypes.c_int64

    @contextlib.contextmanager
    def _hook(output_dir, device_ids):
        import jax
        jax.devices()
        if device_ids:
            ids = (ctypes.c_int64 * len(device_ids))(*device_ids)
            rc = lib.axon_start_nrt_profile(ids, len(device_ids))
        else:
            rc = lib.axon_start_nrt_profile(None, 0)
        if rc != 0:
            raise RuntimeError(f"axon_start_nrt_profile rc={rc}")
        try:
            yield
        finally:
            n = lib.axon_stop_nrt_profile(str(output_dir).encode())
            print(f"profile: {n} file(s) written to {output_dir}")

    set_axon_ntff_profile_hook(_hook)


def _build_nc(dbg=False):
    from contextlib import ExitStack
    from concourse import bass, bacc, mybir, tile

    f32 = mybir.dt.float32
    bf16 = mybir.dt.bfloat16
    i32 = mybir.dt.int32
    AF = mybir.ActivationFunctionType
    ALU = mybir.AluOpType

    nc = bacc.Bacc(
        "TRN2",
        target_bir_lowering=False,
        debug=False,
        enable_asserts=False,
        num_devices=8,
    )

    d_emb = nc.dram_tensor("emb", (V, E), f32, kind="ExternalInput")
    d_u = nc.dram_tensor("u_w", (4 * 128, G4), bf16, kind="ExternalInput")
    d_w = nc.dram_tensor("w_w", (3 * 128, G4), bf16, kind="ExternalInput")
    d_idx = nc.dram_tensor("idx", (128, NB), i32, kind="ExternalInput")
    d_bias_i = nc.dram_tensor("bias_i", (M, T), f32, kind="ExternalInput")
    d_bias_f = nc.dram_tensor("bias_f", (M, T), f32, kind="ExternalInput")
    d_ispad = nc.dram_tensor("ispad", (M, T), f32, kind="ExternalInput")
    d_hinit = nc.dram_tensor("h_init", (32, H), bf16, kind="ExternalInput")
    d_hTinit = nc.dram_tensor("hT_init", (128, 128), bf16, kind="ExternalInput")
    d_cinit = nc.dram_tensor("c_init", (M, H), bf16, kind="ExternalInput")
    d_idbf = nc.dram_tensor("ident_bf", (128, 128), bf16, kind="ExternalInput")
    d_idf32 = nc.dram_tensor("ident_f32", (128, 128), f32, kind="ExternalInput")

    if dbg:
        d_oxw = nc.dram_tensor("o_xw", (M * T, G4), bf16, kind="ExternalOutput")
        d_og = nc.dram_tensor("o_g", (M, G4), bf16, kind="ExternalOutput")
        d_oet = nc.dram_tensor("o_et", (128, 3 * G4), bf16, kind="ExternalOutput")
        d_oh1 = nc.dram_tensor("o_h1", (32, H), bf16, kind="ExternalOutput")
        d_oht1 = nc.dram_tensor("o_ht1", (128, 128), bf16, kind="ExternalOutput")
        d_oc1 = nc.dram_tensor("o_c1", (M, H), bf16, kind="ExternalOutput")
    d_oseq = nc.dram_tensor("o_seq", (M * T, H), bf16, kind="ExternalOutput")
    d_oh = nc.dram_tensor("o_h", (M, H), bf16, kind="ExternalOutput")
    d_oc = nc.dram_tensor("o_c", (M, H), bf16, kind="ExternalOutput")

    with ExitStack() as ctx:
        tc = ctx.enter_context(tile.TileContext(nc))
        const = ctx.enter_context(tc.tile_pool(name="const", bufs=1))

        u_sb = const.tile([128, 4 * G4], bf16, tag="u_sb")
        w_sb = const.tile([128, 3 * G4], bf16, tag="w_sb")
        et_sb = const.tile([128, 3 * G4], bf16, tag="et_sb")
        idx_sb = const.tile([128, NB], i32, tag="idx_sb")
        bias_i_sb = const.tile([M, T], f32, tag="bias_i_sb")
        bias_f_sb = const.tile([M, T], f32, tag="bias_f_sb")
        ispad_sb = const.tile([M, T], f32, tag="ispad_sb")
        idbf_sb = const.tile([128, 128], bf16, tag="idbf_sb")
        idf32_sb = const.tile([128, 128], f32, tag="idf32_sb")
        # state (explicit double buffers where needed)
        cA = const.tile([M, 256], bf16, tag="cA")
        cB = const.tile([M, 256], bf16, tag="cB")
        hA = [const.tile([32, 256], bf16, tag=f"hA{i}", name=f"hA{i}") for i in range(2)]
        hB = [const.tile([32, 256], bf16, tag=f"hB{i}", name=f"hB{i}") for i in range(2)]
        hTA = [const.tile([128, 64], bf16, tag=f"hTA{i}", name=f"hTA{i}") for i in range(2)]
        hTB = [const.tile([128, 64], bf16, tag=f"hTB{i}", name=f"hTB{i}") for i in range(2)]

        # ---- input loads ----
        nc.sync.dma_start(u_sb[:].rearrange("p (k n) -> p k n", k=4),
                          d_u.ap().rearrange("(k p) n -> p k n", p=128))
        nc.sync.dma_start(w_sb[:].rearrange("p (k n) -> p k n", k=3),
                          d_w.ap().rearrange("(k p) n -> p k n", p=128))
        nc.sync.dma_start(idx_sb[:], d_idx[:])
        nc.sync.dma_start(bias_i_sb[:], d_bias_i[:])
        nc.sync.dma_start(bias_f_sb[:], d_bias_f[:])
        nc.sync.dma_start(ispad_sb[:], d_ispad[:])
        nc.sync.dma_start(idbf_sb[:], d_idbf[:])
        nc.sync.dma_start(idf32_sb[:], d_idf32[:])
        nc.sync.dma_start(cA[:], d_cinit[:, 0:256])
        nc.sync.dma_start(cB[:], d_cinit[:, 256:512])
        nc.sync.dma_start(hA[0][:], d_hinit[:, 0:256])
        nc.sync.dma_start(hB[0][:], d_hinit[:, 256:512])
        nc.sync.dma_start(hTA[0][:], d_hTinit[:, 0:64])
        nc.sync.dma_start(hTB[0][:], d_hTinit[:, 64:128])
        nc.vector.memset(hA[1][:], 0.0)
        nc.vector.memset(hB[1][:], 0.0)
        # e^T chunk 2: zero the pad rows 44..63, ones-row at 64 (bias)
        nc.vector.memset(et_sb[32:64, 2 * G4:3 * G4], 0.0)
        nc.vector.memset(et_sb[64:128, 2 * G4:3 * G4], 1.0)

        # preload the sigmoid/tanh ACT table set early (off the scan path)
        warm = const.tile([1, 8], f32, tag="warm")
        nc.scalar.activation(warm[:], idf32_sb[0:1, 0:8], AF.Sigmoid)

        dramp = ctx.enter_context(tc.tile_pool(name="dramp", bufs=1, space="DRAM"))
        xw_d = dramp.tile([M * T, G4], bf16, tag="xw_d")

        # ---- phase 1: gather + transpose + xW precompute ----
        with tc.tile_pool(name="p1", bufs=3) as p1, \
             tc.tile_pool(name="p1ps", bufs=2, space="PSUM") as p1ps, \
             tc.tile_pool(name="p1ps2", bufs=1, space="PSUM") as p1ps2:
            for j in range(NB):
                etok = p1.tile([128, 304], f32, tag="etok")
                nc.gpsimd.indirect_dma_start(
                    out=etok[:, 0:E],
                    out_offset=None,
                    in_=d_emb[:],
                    in_offset=bass.IndirectOffsetOnAxis(ap=idx_sb[:, j:j + 1], axis=0),
                )
                for c in range(3):
                    cw = 128 if c < 2 else 44
                    tp = p1ps.tile([128, 128], f32, tag="tp")
                    nc.tensor.transpose(
                        out=tp[0:cw, 0:128],
                        in_=etok[0:128, c * 128:c * 128 + cw],
                        identity=idf32_sb[:],
                    )
                    nc.vector.tensor_copy(
                        et_sb[0:cw, c * G4 + j * 128:c * G4 + j * 128 + 128],
                        tp[0:cw, 0:128],
                    )
                xps = p1ps2.tile([128, G4], f32, tag="xps")
                for b4 in range(4):
                    for c in range(3):
                        kc = EK[c]
                        nc.tensor.matmul(
                            out=xps[:, b4 * 512:(b4 + 1) * 512],
                            lhsT=et_sb[0:kc, c * G4 + j * 128:c * G4 + j * 128 + 128],
                            rhs=w_sb[0:kc, c * G4 + b4 * 512:c * G4 + (b4 + 1) * 512],
                            start=(c == 0),
                            stop=(c == 2),
                        )
                xst = p1.tile([128, G4], bf16, tag="xst")
                if j % 2 == 0:
                    nc.vector.tensor_copy(xst[:], xps[:])
                else:
                    nc.scalar.copy(xst[:], xps[:])
                nc.sync.dma_start(xw_d[j * 128:(j + 1) * 128, :], xst[:])
                if dbg:
                    nc.sync.dma_start(d_oxw[j * 128:(j + 1) * 128, :], xst[:])

        # ---- phase 2: the scan ----
        zpA = ctx.enter_context(tc.tile_pool(name="zpA", bufs=2, space="PSUM"))
        zpB = ctx.enter_context(tc.tile_pool(name="zpB", bufs=2, space="PSUM"))
        gp = ctx.enter_context(tc.tile_pool(name="gp", bufs=2))
        xwp = ctx.enter_context(tc.tile_pool(name="xwp", bufs=4))
        tp2 = ctx.enter_context(tc.tile_pool(name="tp2", bufs=3))

        for t in range(T):
            r, w = t % 2, (t + 1) % 2

            zA = zpA.tile([32, 1024], mybir.dt.float32, tag="zA")
            zB = zpB.tile([32, 1024], mybir.dt.float32, tag="zB")
            xws = xwp.tile([M, G4], bf16, tag="xws")
            nc.sync.dma_start(xws[:], xw_d[t * M:(t + 1) * M, :])

            # PE: inject xW_t (identity matmul, start=True), then U-matmuls
            # K-major so each K chunk is gated only on its h^T chunk.
            for half, z in ((0, zA), (1, zB)):
                for b2 in range(2):
                    nc.tensor.matmul(
                        out=z[:, b2 * 512:(b2 + 1) * 512],
                        lhsT=idbf_sb[0:M, 0:32],
                        rhs=xws[0:M,
                                half * 1024 + b2 * 512:
                                half * 1024 + (b2 + 1) * 512],
                        start=True, stop=False,
                    )
            for k in range(4):
                hts = hTA[r] if k < 2 else hTB[r]
                col = (k % 2) * 32
                for half, z in ((0, zA), (1, zB)):
                    for b2 in range(2):
                        nc.tensor.matmul(
                            out=z[:, b2 * 512:(b2 + 1) * 512],
                            lhsT=hts[:, col:col + 32],
                            rhs=u_sb[:, k * G4 + half * 1024 + b2 * 512:
                                     k * G4 + half * 1024 + (b2 + 1) * 512],
                            start=False, stop=(k == 3),
                        )

            # ACT: gates.  Layout per half: [i(256) | o(256) | f(256) | g(256)]
            gA = gp.tile([M, 1024], bf16, tag="gA")
            gB = gp.tile([M, 1024], bf16, tag="gB")
            for z, g in ((zA, gA), (zB, gB)):
                nc.scalar.activation(g[:, 0:512], z[0:M, 0:512], AF.Sigmoid,
                                     bias=bias_i_sb[:, t:t + 1])
                nc.scalar.activation(g[:, 512:768], z[0:M, 512:768], AF.Sigmoid,
                                     bias=bias_f_sb[:, t:t + 1])
                nc.scalar.activation(g[:, 768:1024], z[0:M, 768:1024], AF.Tanh)

            if dbg and t == 0:
                nc.sync.dma_start(d_og[:, 0:1024], gA[:])
                nc.sync.dma_start(d_og[:, 1024:2048], gB[:])
                nc.sync.dma_start(d_oet[:], et_sb[:])

            # mask keep-terms, cheap early TSP (2x/4x single-src mode)
            hkA = tp2.tile([M, 256], bf16, tag="hkA")
            hkB = tp2.tile([M, 256], bf16, tag="hkB")
            nc.vector.tensor_scalar_mul(hkA[:], hA[r][0:M, :], ispad_sb[:, t:t + 1])
            nc.vector.tensor_scalar_mul(hkB[:], hB[r][0:M, :], ispad_sb[:, t:t + 1])

            # c update (all DVE; masked rows: i'=0, f'=1 -> c carries)
            tcs = []
            for g, c_t, nm in ((gA, cA, "A"), (gB, cB, "B")):
                fc = tp2.tile([M, 256], bf16, tag=f"fc{nm}")
                ig = tp2.tile([M, 256], bf16, tag=f"ig{nm}")
                nc.vector.tensor_tensor(fc[:], g[:, 512:768], c_t[:], op=ALU.mult)
                nc.vector.tensor_tensor(ig[:], g[:, 0:256], g[:, 768:1024], op=ALU.mult)
                nc.vector.tensor_tensor(c_t[:], fc[:], ig[:], op=ALU.add)
                tcv = tp2.tile([M, 256], bf16, tag=f"tc{nm}")
                nc.scalar.activation(tcv[:], c_t[:], AF.Tanh)
                tcs.append(tcv)
            otA = tp2.tile([M, 256], bf16, tag="otA")
            otB = tp2.tile([M, 256], bf16, tag="otB")
            nc.vector.tensor_tensor(otA[:], gA[:, 256:512], tcs[0][:], op=ALU.mult)
            nc.vector.tensor_tensor(hA[w][0:M, :], otA[:], hkA[:], op=ALU.add)
            nc.vector.tensor_tensor(otB[:], gB[:, 256:512], tcs[1][:], op=ALU.mult)
            nc.vector.tensor_tensor(hB[w][0:M, :], otB[:], hkB[:], op=ALU.add)

            # DVE: h^T via 32x32 block transposes (rows 8..31 are zero).
            # h cols are ILV-interleaved so each op is a contiguous [32,64].
            for hsrc, hdst in ((hA[w], hTA[w]), (hB[w], hTB[w])):
                for j in range(4):
                    nc.vector.transpose(
                        hdst[32 * j:32 * j + 32, :],
                        hsrc[:, 64 * j:64 * j + 64],
                    )

            if dbg and t == 0:
                nc.sync.dma_start(d_oh1[:, 0:256], hA[w][:])
                nc.sync.dma_start(d_oh1[:, 256:512], hB[w][:])
                nc.sync.dma_start(d_oht1[:, 0:64], hTA[w][:])
                nc.sync.dma_start(d_oht1[:, 64:128], hTB[w][:])
                nc.sync.dma_start(d_oc1[:, 0:256], cA[:])
                nc.sync.dma_start(d_oc1[:, 256:512], cB[:])

            # stream the step's output
            nc.sync.dma_start(d_oseq[t * M:(t + 1) * M, 0:256], hA[w][0:M, :])
            nc.sync.dma_start(d_oseq[t * M:(t + 1) * M, 256:512], hB[w][0:M, :])

        # final states
        wf = T % 2
        nc.sync.dma_start(d_oh[:, 0:256], hA[wf][0:M, :])
        nc.sync.dma_start(d_oh[:, 256:512], hB[wf][0:M, :])
        nc.sync.dma_start(d_oc[:, 0:256], cA[:])
        nc.sync.dma_start(d_oc[:, 256:512], cB[:])

    nc.compile()
    return nc


def _prep_core(x_sh, h0_sh, c0_sh, W, U, b):
    """Host-side layout prep for one core.  x_sh [M,T] int32 (already
    time-reversed for backward cores), h0/c0 [M,H] f32, W [E,4H], U [H,4H],
    b [4H] f32.  Returns the per-core in_map."""
    x_sh = np.ascontiguousarray(x_sh).astype(np.int32)
    # gather indices, token-major (token = t*M + b)
    flat = x_sh.T.reshape(-1)                       # [T*M]
    idx = flat.reshape(NB, 128).T.copy()            # [128, NB]
    ispad = (x_sh == 0).astype(np.float32)          # [M, T]
    bias_i = ispad * (-BIG)
    bias_f = ispad * BIG

    # W augmented with bias row, padded to 384 rows, columns permuted
    W_aug = np.zeros((384, G4), np.float32)
    W_aug[0:E] = W
    W_aug[320] = b        # chunk-2 row 64, pairs with the e^T ones-row
    W_perm = W_aug[:, _PERM].astype(BF16)
    U_perm = U[:, _PERM].astype(BF16)

    h_init = np.zeros((32, H), np.float32)
    h_init[0:M] = h0_sh[:, _PERM_H]
    hT_init = np.zeros((128, 128), np.float32)
    for k in range(4):
        hT_init[:, 32 * k:32 * k + M] = h0_sh[:, 128 * k:128 * (k + 1)].T

    return {
        "u_w": np.ascontiguousarray(U_perm),
        "w_w": np.ascontiguousarray(W_perm),
        "idx": np.ascontiguousarray(idx),
        "bias_i": np.ascontiguousarray(bias_i),
        "bias_f": np.ascontiguousarray(bias_f),
        "ispad": np.ascontiguousarray(ispad),
        "h_init": h_init.astype(BF16),
        "hT_init": hT_init.astype(BF16),
        "c_init": c0_sh[:, _PERM_H].astype(BF16),
        "ident_bf": np.eye(128, dtype=BF16),
        "ident_f32": np.eye(128, dtype=np.float32),
    }


def kernel(x, h0_fwd, c0_fwd, h0_bwd, c0_bwd, emb,
           W_fwd, U_fwd, b_fwd, W_bwd, U_bwd, b_bwd):
    global _COMPILED, LAST_RESULTS
    from concourse.bass_utils import run_bass_kernel_spmd

    x = np.asarray(x, np.int32)
    emb = np.ascontiguousarray(np.asarray(emb, np.float32))
    h0_fwd = np.asarray(h0_fwd, np.float32)
    c0_fwd = np.asarray(c0_fwd, np.float32)
    h0_bwd = np.asarray(h0_bwd, np.float32)
    c0_bwd = np.asarray(c0_bwd, np.float32)
    W_fwd, U_fwd, b_fwd = (np.asarray(a, np.float32) for a in (W_fwd, U_fwd, b_fwd))
    W_bwd, U_bwd, b_bwd = (np.asarray(a, np.float32) for a in (W_bwd, U_bwd, b_bwd))

    if _COMPILED is None:
        _COMPILED = _build_nc()
    nc = _COMPILED

    in_maps = []
    for core in range(8):
        if core < 4:
            rows = slice(8 * core, 8 * core + 8)
            m = _prep_core(x[rows], h0_fwd[rows], c0_fwd[rows], W_fwd, U_fwd, b_fwd)
        else:
            rows = slice(8 * (core - 4), 8 * (core - 4) + 8)
            m = _prep_core(x[rows, ::-1], h0_bwd[rows], c0_bwd[rows],
                           W_bwd, U_bwd, b_bwd)
        m["emb"] = emb
        in_maps.append(m)

    trace = bool(int(os.environ.get("KERNEL_TRACE", "0")))
    if trace:
        _install_ntff_hook_shim()
    res = run_bass_kernel_spmd(
        nc, in_maps, core_ids=list(range(8)), trace=trace,
    )
    LAST_RESULTS = res

    output = np.zeros((B, T, 2 * H), np.float32)
    h_f = np.zeros((B, H), np.float32)
    h_b = np.zeros((B, H), np.float32)
    c_f = np.zeros((B, H), np.float32)
    c_b = np.zeros((B, H), np.float32)
    for core in range(8):
        r = res.results[core]
        seq = np.asarray(r["o_seq"]).astype(np.float32)      # [T*M, H]
        seq = seq.reshape(T, M, H).transpose(1, 0, 2)[:, :, _INV_H]
        hT_ = np.asarray(r["o_h"]).astype(np.float32)[:, _INV_H]
        cT_ = np.asarray(r["o_c"]).astype(np.float32)[:, _INV_H]
        if core < 4:
            rows = slice(8 * core, 8 * core + 8)
            output[rows, :, 0:H] = seq
            h_f[rows] = hT_
            c_f[rows] = cT_
        else:
            rows = slice(8 * (core - 4), 8 * (core - 4) + 8)
            output[rows, :, H:2 * H] = seq[:, ::-1, :]
            h_b[rows] = hT_
            c_b[rows] = cT_
    return output, h_f, h_b, c_f, c_b
